# revision 33
# baseline (speedup 1.0000x reference)
"""Causal self-attention (S=2048, D=1024, H=16) on 8 Trainium2 NeuronCores.

Sharding: tensor-parallel over heads. Core c owns heads 2c, 2c+1:
  - computes qT/kT/vT for its 128 qkv-columns from the full hidden_states
    (contraction layouts; vT is PE-transposed back to natural [s, j]),
  - runs causal attention for its 2 heads (attT = K.Q^T blocks, exp via
    ScalarE, denominators via a ones-column in the PV matmul),
  - projects each head against its W_proj row-slice and fuses the softmax
    normalization into the projection epilogue (per-partition 1/den scales,
    denominators PE-transposed so one wide reciprocal covers all of them),
  - outputs a partial [S, D] product; the host sums the 8 partials and
    adds b_proj.

The bulk matmuls run in bf16 (1 cycle/row); the unnormalized attention
outputs, denominators and the final projection stay in float32r (fp32
storage, single-pass PE). Phase 2 is software-pipelined (logits of group g
overlap the PV matmuls of group g-1) to keep the PE dense and the HAM
clock warm; dummy matmuls paced by the input DMA keep the clock warm
during the initial load.
"""

import math
from contextlib import ExitStack

import numpy as np

import concourse.bacc as bacc
import concourse.mybir as mybir
import concourse.tile as tile
from concourse.bass_utils import run_bass_kernel_spmd

S, D, H = 2048, 1024, 16
HS = D // H  # 64 head size
P = 128
NCORES = 8
HPC = H // NCORES  # 2 heads per core
CD = HPC * HS  # 128 per-core head dims
KO = D // P  # 8 contraction tiles for the projections
NQC = S // 512  # 4 query chunks
NSC = S // P  # 16 sequence chunks of 128
SCALE = 1.0 / math.sqrt(S)

F32 = mybir.dt.float32
F32R = mybir.dt.float32r
BF16 = mybir.dt.bfloat16

try:
    import ml_dtypes

    NP_BF16 = ml_dtypes.bfloat16
except ImportError:  # pragma: no cover
    NP_BF16 = None


def _build():
    nc = bacc.Bacc(
        "TRN2", target_bir_lowering=False, debug=False, num_devices=NCORES
    )

    hsT = nc.dram_tensor("hsT", [D, S], BF16, kind="ExternalInput")
    w_qkv = nc.dram_tensor("w_qkv", [D, 3 * P], BF16, kind="ExternalInput")
    b_qkv = nc.dram_tensor("b_qkv", [P, 3], F32, kind="ExternalInput")
    w_p = nc.dram_tensor("w_p", [CD, D], BF16, kind="ExternalInput")
    msk = nc.dram_tensor("msk", [P, 896], BF16, kind="ExternalInput")
    iden_b = nc.dram_tensor("iden_b", [P, P], BF16, kind="ExternalInput")
    vones = nc.dram_tensor("vones", [P, NSC], BF16, kind="ExternalInput")
    ones_r = nc.dram_tensor("ones_r", [1, P], F32R, kind="ExternalInput")
    out = nc.dram_tensor("out", [S, D], BF16, kind="ExternalOutput")

    with (
        tile.TileContext(nc) as tc,
        ExitStack() as ctx,
        nc.allow_low_precision(reason="bf16/float32r matmul pipeline"),
    ):
        const = ctx.enter_context(tc.tile_pool(name="const", bufs=1))
        work = ctx.enter_context(tc.tile_pool(name="work", bufs=2))
        pp = ctx.enter_context(tc.tile_pool(name="pp", bufs=1, space="PSUM"))

        def psA(name):  # generic 2-bank matmul target, 3 slots
            return pp.tile([P, 2, 512], F32, tag="A", bufs=3, name=name)

        # ---- loads: identity first (pre-warm), then per-o weight+hsT chunks
        identb = const.tile([P, P], BF16, tag="identb", name="identb")
        nc.sync.dma_start(out=identb, in_=iden_b.ap())
        onesr_sb = const.tile([1, P], F32R, tag="onesr", name="onesr_sb")
        nc.sync.dma_start(out=onesr_sb, in_=ones_r.ap())
        bqkv_sb = const.tile([P, 3], F32, tag="bqkv", name="bqkv_sb")
        nc.sync.dma_start(out=bqkv_sb, in_=b_qkv.ap())

        hsT_sb = const.tile([P, KO, S], BF16, tag="hsT", name="hsT_sb")
        wqkv_sb = const.tile([P, KO, 3 * P], BF16, tag="wqkv", name="wqkv_sb")
        for o in range(KO):
            nc.gpsimd.dma_start(
                out=wqkv_sb[:, o, :], in_=w_qkv.ap()[o * P : (o + 1) * P, :]
            )
            nc.sync.dma_start(
                out=hsT_sb[:, o, 0:512], in_=hsT.ap()[o * P : (o + 1) * P, 0:512]
            )
        wp_sb = const.tile([P, D], BF16, tag="wp", name="wp_sb")
        nc.sync.dma_start(out=wp_sb, in_=w_p.ap())
        msk_sb = const.tile([P, 896], BF16, tag="msk", name="msk_sb")
        nc.sync.dma_start(out=msk_sb, in_=msk.ap())
        v_sb = []
        for h in range(HPC):
            vt = const.tile([P, NSC, HS + 1], BF16, tag=f"v{h}", name=f"v{h}_sb")
            nc.sync.dma_start(out=vt[:, :, HS], in_=vones.ap())
            v_sb.append(vt)
        for n in range(1, NQC):
            for o in range(KO):
                eng = nc.sync if (n * KO + o) % 2 == 0 else nc.gpsimd
                eng.dma_start(
                    out=hsT_sb[:, o, n * 512 : (n + 1) * 512],
                    in_=hsT.ap()[o * P : (o + 1) * P, n * 512 : (n + 1) * 512],
                )

        qkT_sb = const.tile([P, 2, S], BF16, tag="qkT", name="qkT_sb")
        vT_sb = const.tile([P, S], BF16, tag="vT", name="vT_sb")
        u2_sb = [
            const.tile([P, 512], F32R, tag=f"u2_{qc}", name=f"u2_{qc}")
            for qc in range(NQC)
        ]
        u2n_sb = [
            const.tile([P, 512], BF16, tag=f"u2n_{qc}", name=f"u2n_{qc}")
            for qc in range(NQC)
        ]
        den_sb = {
            (qc, h): const.tile([1, 512], F32, tag=f"den_{qc}_{h}", name=f"den_{qc}_{h}")
            for qc in range(NQC)
            for h in range(HPC)
        }

        # ---- pre-warm the PE clock while the DMAs stream ---------------------
        # each burst consumes a freshly-arrived hsT chunk so the bursts are
        # spread across the load instead of back-to-back at t=0
        ps_w = psA("ps_w")
        for o in range(KO):
            for rep in range(6):
                nc.tensor.matmul(
                    ps_w[:, 0, :],
                    lhsT=identb,
                    rhs=hsT_sb[:, o, 0:512],
                    start=True,
                    stop=True,
                )

        # ---- phase 1: qT, kT, vT ([j, s] layout) + v transposes -------------
        # emitted per 512-chunk, interleaved with the attention chunks that
        # consume them (phase-1 is DMA-paced, phase-2 is ScalarE-bound; the
        # PE fills both phases' gaps)
        def emit_p1(n):
            for m in range(3):
                ps_qkv = psA("ps_qkv")[:, 0, :]
                for o in range(KO):
                    nc.tensor.matmul(
                        ps_qkv,
                        lhsT=wqkv_sb[:, o, m * P : (m + 1) * P],
                        rhs=hsT_sb[:, o, n * 512 : (n + 1) * 512],
                        start=(o == 0),
                        stop=(o == KO - 1),
                    )
                dst = (
                    qkT_sb[:, m, n * 512 : (n + 1) * 512]
                    if m < 2
                    else vT_sb[:, n * 512 : (n + 1) * 512]
                )
                nc.vector.tensor_scalar_add(
                    out=dst, in0=ps_qkv, scalar1=bqkv_sb[:, m : m + 1]
                )
            # transpose this n-chunk of vT into natural v layout
            for sc in range(4 * n, 4 * n + 4):
                ps_t = pp.tile([P, P], BF16, tag="A", bufs=3, name="ps_t")
                nc.tensor.transpose(ps_t, vT_sb[:, sc * P : (sc + 1) * P], identb)
                for h in range(HPC):
                    nc.vector.tensor_copy(
                        out=v_sb[h][:, sc, 0:HS], in_=ps_t[:, h * HS : (h + 1) * HS]
                    )

        emit_p1(0)

        def emit_norm(qc):
            # 1/s rows via exp(-ln(s)), broadcast via K=1 matmuls, one
            # normalize multiply per head
            rb_ps = psA("ps_rb")
            for h in range(HPC):
                lg = work.tile([1, 512], F32, tag=f"lg{h}", bufs=2, name="lg")
                nc.scalar.activation(
                    out=lg,
                    in_=den_sb[(qc, h)],
                    func=mybir.ActivationFunctionType.Ln,
                )
                rrow = work.tile([1, 512], F32R, tag=f"rr{h}", bufs=2, name="rrow")
                nc.scalar.activation(
                    out=rrow,
                    in_=lg,
                    func=mybir.ActivationFunctionType.Exp,
                    scale=-1.0,
                )
                nc.tensor.matmul(
                    rb_ps[0:HS, h, :],
                    lhsT=onesr_sb[:, 0:HS],
                    rhs=rrow,
                    start=True,
                    stop=True,
                )
            for h in range(HPC):
                nc.vector.tensor_mul(
                    out=u2n_sb[qc][h * HS : (h + 1) * HS, :],
                    in0=u2_sb[qc][h * HS : (h + 1) * HS, :],
                    in1=rb_ps[0:HS, h, :],
                )

        # ---- phase 2: causal attention, software-pipelined ------------------
        for qc in range(NQC):
            ps_o = [
                pp.tile([P, 512], F32, tag="O", bufs=2, name=f"ps_o{h}")
                for h in range(HPC)
            ]
            nkb = 4 * (qc + 1)  # 128-wide key blocks in the causal span
            ngrp = nkb // 2

            def emit_pv(pend, nkb=nkb, ps_o=ps_o):
                pes, kbs, f0 = pend
                for h in range(HPC):
                    for j, kb in enumerate(kbs):
                        nc.tensor.matmul(
                            ps_o[h][0 : HS + 1, f0:512],
                            lhsT=v_sb[h][:, kb, :],
                            rhs=pes[h][:, j, f0:512],
                            start=(kb == 0),
                            stop=(kb == nkb - 1),
                        )

            pending = None  # exp'd logits awaiting their PV matmuls
            for g in range(ngrp):
                kbs = [2 * g, 2 * g + 1]
                # last group covers only the causal upper half of the q range
                f0 = 256 if g == ngrp - 1 else 0
                # logits for both heads, adjacent for row-group packing
                ps_att = [psA(f"ps_att{h}") for h in range(HPC)]
                for j, kb in enumerate(kbs):
                    for h in range(HPC):
                        nc.tensor.matmul(
                            ps_att[h][:, j, f0:512],
                            lhsT=qkT_sb[h * HS : (h + 1) * HS, 1, kb * P : (kb + 1) * P],
                            rhs=qkT_sb[h * HS : (h + 1) * HS, 0, qc * 512 + f0 : (qc + 1) * 512],
                            start=True,
                            stop=True,
                        )
                if pending is not None:
                    emit_pv(pending)
                pes = []
                for h in range(HPC):
                    p_exp = work.tile(
                        [P, 2, 512], BF16, tag=f"pe{h}", bufs=4, name="p_exp"
                    )
                    nc.scalar.activation(
                        out=p_exp[:, :, f0:512],
                        in_=ps_att[h][:, :, f0:512],
                        func=mybir.ActivationFunctionType.Exp,
                        scale=SCALE,
                    )
                    for j, kb in enumerate(kbs):
                        jj = kb - 4 * qc
                        if jj >= 0:  # diagonal block: causal 0/1 mask
                            off = 384 - 128 * jj
                            nc.vector.tensor_mul(
                                out=p_exp[:, j, f0:512],
                                in0=p_exp[:, j, f0:512],
                                in1=msk_sb[:, off + f0 : off + 512],
                            )
                    pes.append(p_exp)
                pending = (pes, kbs, f0)
            emit_pv(pending)

            # stash unnormalized head outputs + denominator rows; frees PSUM
            for h in range(HPC):
                nc.vector.tensor_copy(
                    out=u2_sb[qc][h * HS : (h + 1) * HS, :], in_=ps_o[h][0:HS, :]
                )
                nc.vector.tensor_copy(
                    out=den_sb[(qc, h)], in_=ps_o[h][HS : HS + 1, :]
                )
            if qc + 1 < NQC:
                emit_p1(qc + 1)
            if qc >= 1:
                emit_norm(qc - 1)

        # ---- denominator reciprocals as rows: 1/s = exp(-ln(s)) on ScalarE,
        # broadcast to all partitions with K=1 matmuls, then one multiply
        # pre-normalizes each 512-wide chunk of the attention output
        # ---- phase 3: projection over both heads (K=128), pipelined ---------
        def emit_p3_mm(sc):
            qc = sc // 4
            f = sc % 4
            slot = psA("ps_p3")
            for dc in range(2):
                nc.tensor.matmul(
                    slot[:, dc, :],
                    lhsT=u2n_sb[qc][:, f * P : (f + 1) * P],
                    rhs=wp_sb[:, dc * 512 : (dc + 1) * 512],
                    start=True,
                    stop=True,
                )
            return slot

        def emit_p3_epi(sc, slot):
            out_t = work.tile([P, 2, 512], BF16, tag="out", bufs=3, name="out_t")
            nc.vector.tensor_copy(out=out_t, in_=slot)
            eng = nc.sync if sc % 2 == 0 else nc.gpsimd
            eng.dma_start(
                out=out.ap()[sc * P : (sc + 1) * P, :],
                in_=out_t.rearrange("p a b -> p (a b)"),
            )

        p3q = []
        for sc in range(NSC):
            p3q.append((sc, emit_p3_mm(sc)))
            if sc == 1:
                emit_norm(3)
            if len(p3q) > 1:
                emit_p3_epi(*p3q.pop(0))
        for item in p3q:
            emit_p3_epi(*item)

    nc.compile()
    return nc


_NC = None


def _get_nc():
    global _NC
    if _NC is None:
        _NC = _build()
    return _NC


def prepare_inputs(hidden_states, W_attn, b_attn, W_proj, b_proj):
    hs = np.asarray(hidden_states, dtype=np.float32)
    Wa = np.asarray(W_attn, dtype=np.float32)
    ba = np.asarray(b_attn, dtype=np.float32)
    Wp = np.asarray(W_proj, dtype=np.float32)

    hsT = np.ascontiguousarray(hs.T).astype(NP_BF16)
    pcol = np.arange(P)[:, None]
    ccol = np.arange(896)[None, :]
    msk = (pcol <= ccol - 384).astype(NP_BF16)

    in_maps = []
    for c in range(NCORES):
        q0 = c * CD
        wq = Wa[:, q0 : q0 + CD]
        wk = Wa[:, D + q0 : D + q0 + CD]
        wv = Wa[:, 2 * D + q0 : 2 * D + q0 + CD]
        bq = ba[q0 : q0 + CD]
        bk = ba[D + q0 : D + q0 + CD]
        bv = ba[2 * D + q0 : 2 * D + q0 + CD]
        in_maps.append(
            {
                "hsT": hsT,
                "w_qkv": np.ascontiguousarray(
                    np.concatenate([wq, wk, wv], axis=1)
                ).astype(NP_BF16),
                "b_qkv": np.ascontiguousarray(np.stack([bq, bk, bv], axis=1)).astype(
                    np.float32
                ),
                "w_p": np.ascontiguousarray(Wp[q0 : q0 + CD, :]).astype(NP_BF16),
                "msk": msk,
                "iden_b": np.eye(P).astype(NP_BF16),
                "ones_r": np.ones((1, P), dtype=np.float32),
                "vones": np.ones((P, NSC)).astype(NP_BF16),
            }
        )
    return in_maps


def run(inputs, trace=False):
    """Build+run the sharded kernel. Returns (full_output, BassKernelResults)."""
    in_maps = prepare_inputs(**inputs)
    nc = _get_nc()
    res = run_bass_kernel_spmd(
        nc, in_maps, core_ids=list(range(NCORES)), trace=trace
    )
    acc = np.zeros((S, D), dtype=np.float32)
    for c in range(NCORES):
        acc += res.results[c]["out"].astype(np.float32)
    acc += np.asarray(inputs["b_proj"], dtype=np.float32)
    return acc, res


def kernel(**inputs):
    out, _ = run(inputs, trace=False)
    return out


# revision 34
# speedup vs baseline: 1.1287x; 1.1287x over previous
"""Causal self-attention (S=2048, D=1024, H=16) on 8 Trainium2 NeuronCores.

Sharding: tensor-parallel over heads. Core c owns heads 2c, 2c+1:
  - computes qT/kT/vT for its 128 qkv-columns from the full hidden_states
    (contraction layouts; vT is PE-transposed back to natural [s, j]),
  - runs causal attention for its 2 heads (attT = K.Q^T blocks, exp via
    ScalarE, denominators via a ones-column in the PV matmul),
  - projects each head against its W_proj row-slice and fuses the softmax
    normalization into the projection epilogue (per-partition 1/den scales,
    denominators PE-transposed so one wide reciprocal covers all of them),
  - outputs a partial [S, D] product; the host sums the 8 partials and
    adds b_proj.

The bulk matmuls run in bf16 (1 cycle/row); the unnormalized attention
outputs, denominators and the final projection stay in float32r (fp32
storage, single-pass PE). Phase 2 is software-pipelined (logits of group g
overlap the PV matmuls of group g-1) to keep the PE dense and the HAM
clock warm; dummy matmuls paced by the input DMA keep the clock warm
during the initial load.
"""

import math
from contextlib import ExitStack

import numpy as np

import concourse.bacc as bacc
import concourse.mybir as mybir
import concourse.tile as tile
from concourse.bass_utils import run_bass_kernel_spmd

S, D, H = 2048, 1024, 16
HS = D // H  # 64 head size
P = 128
NCORES = 8
HPC = H // NCORES  # 2 heads per core
CD = HPC * HS  # 128 per-core head dims
KO = D // P  # 8 contraction tiles for the projections
NQC = S // 512  # 4 query chunks
NSC = S // P  # 16 sequence chunks of 128
SCALE = 1.0 / math.sqrt(S)

F32 = mybir.dt.float32
F32R = mybir.dt.float32r
BF16 = mybir.dt.bfloat16

try:
    import ml_dtypes

    NP_BF16 = ml_dtypes.bfloat16
except ImportError:  # pragma: no cover
    NP_BF16 = None


def _build():
    nc = bacc.Bacc(
        "TRN2", target_bir_lowering=False, debug=False, num_devices=NCORES
    )

    hsT = nc.dram_tensor("hsT", [D, S], BF16, kind="ExternalInput")
    w_qkv = nc.dram_tensor("w_qkv", [D, 3 * P], BF16, kind="ExternalInput")
    b_qkv = nc.dram_tensor("b_qkv", [P, 3], F32, kind="ExternalInput")
    w_p = nc.dram_tensor("w_p", [CD, D], F32R, kind="ExternalInput")
    msk = nc.dram_tensor("msk", [P, 896], BF16, kind="ExternalInput")
    iden_b = nc.dram_tensor("iden_b", [P, P], BF16, kind="ExternalInput")
    vones = nc.dram_tensor("vones", [P, NSC], BF16, kind="ExternalInput")
    ones_r = nc.dram_tensor("ones_r", [1, P], F32R, kind="ExternalInput")
    out = nc.dram_tensor("out", [S, D], F32, kind="ExternalOutput")

    with (
        tile.TileContext(nc) as tc,
        ExitStack() as ctx,
        nc.allow_low_precision(reason="bf16/float32r matmul pipeline"),
    ):
        const = ctx.enter_context(tc.tile_pool(name="const", bufs=1))
        work = ctx.enter_context(tc.tile_pool(name="work", bufs=2))
        pp = ctx.enter_context(tc.tile_pool(name="pp", bufs=1, space="PSUM"))

        def psA(name):  # generic 2-bank matmul target, 3 slots
            return pp.tile([P, 2, 512], F32, tag="A", bufs=3, name=name)

        # ---- loads: identity first (pre-warm), then per-o weight+hsT chunks
        identb = const.tile([P, P], BF16, tag="identb", name="identb")
        nc.sync.dma_start(out=identb, in_=iden_b.ap())
        onesr_sb = const.tile([1, P], F32R, tag="onesr", name="onesr_sb")
        nc.sync.dma_start(out=onesr_sb, in_=ones_r.ap())
        bqkv_sb = const.tile([P, 3], F32, tag="bqkv", name="bqkv_sb")
        nc.sync.dma_start(out=bqkv_sb, in_=b_qkv.ap())

        hsT_sb = const.tile([P, KO, S], BF16, tag="hsT", name="hsT_sb")
        wqkv_sb = const.tile([P, KO, 3 * P], BF16, tag="wqkv", name="wqkv_sb")
        for o in range(KO):
            nc.gpsimd.dma_start(
                out=wqkv_sb[:, o, :], in_=w_qkv.ap()[o * P : (o + 1) * P, :]
            )
            nc.sync.dma_start(
                out=hsT_sb[:, o, 0:512], in_=hsT.ap()[o * P : (o + 1) * P, 0:512]
            )
        wp_sb = const.tile([P, D], F32R, tag="wp", name="wp_sb")
        nc.sync.dma_start(out=wp_sb, in_=w_p.ap())
        msk_sb = const.tile([P, 896], BF16, tag="msk", name="msk_sb")
        nc.sync.dma_start(out=msk_sb, in_=msk.ap())
        v_sb = []
        for h in range(HPC):
            vt = const.tile([P, NSC, HS + 1], BF16, tag=f"v{h}", name=f"v{h}_sb")
            nc.sync.dma_start(out=vt[:, :, HS], in_=vones.ap())
            v_sb.append(vt)
        for n in range(1, NQC):
            for o in range(KO):
                eng = nc.sync if (n * KO + o) % 2 == 0 else nc.gpsimd
                eng.dma_start(
                    out=hsT_sb[:, o, n * 512 : (n + 1) * 512],
                    in_=hsT.ap()[o * P : (o + 1) * P, n * 512 : (n + 1) * 512],
                )

        qkT_sb = const.tile([P, 2, S], BF16, tag="qkT", name="qkT_sb")
        vT_sb = const.tile([P, S], BF16, tag="vT", name="vT_sb")
        u2_sb = [
            const.tile([P, 512], F32R, tag=f"u2_{qc}", name=f"u2_{qc}")
            for qc in range(NQC)
        ]
        u2n_sb = [
            const.tile([P, 512], F32R, tag=f"u2n_{qc}", name=f"u2n_{qc}")
            for qc in range(NQC)
        ]
        den_sb = {
            (qc, h): const.tile([1, 512], F32, tag=f"den_{qc}_{h}", name=f"den_{qc}_{h}")
            for qc in range(NQC)
            for h in range(HPC)
        }

        # ---- pre-warm the PE clock while the DMAs stream ---------------------
        # each burst consumes a freshly-arrived hsT chunk so the bursts are
        # spread across the load instead of back-to-back at t=0
        ps_w = psA("ps_w")
        for o in range(KO):
            for rep in range(6):
                nc.tensor.matmul(
                    ps_w[:, 0, :],
                    lhsT=identb,
                    rhs=hsT_sb[:, o, 0:512],
                    start=True,
                    stop=True,
                )

        # ---- phase 1: qT, kT, vT ([j, s] layout) + v transposes -------------
        # emitted per 512-chunk, interleaved with the attention chunks that
        # consume them (phase-1 is DMA-paced, phase-2 is ScalarE-bound; the
        # PE fills both phases' gaps)
        def emit_p1(n):
            for m in range(3):
                ps_qkv = psA("ps_qkv")[:, 0, :]
                for o in range(KO):
                    nc.tensor.matmul(
                        ps_qkv,
                        lhsT=wqkv_sb[:, o, m * P : (m + 1) * P],
                        rhs=hsT_sb[:, o, n * 512 : (n + 1) * 512],
                        start=(o == 0),
                        stop=(o == KO - 1),
                    )
                dst = (
                    qkT_sb[:, m, n * 512 : (n + 1) * 512]
                    if m < 2
                    else vT_sb[:, n * 512 : (n + 1) * 512]
                )
                nc.vector.tensor_scalar_add(
                    out=dst, in0=ps_qkv, scalar1=bqkv_sb[:, m : m + 1]
                )
            # transpose this n-chunk of vT into natural v layout
            for sc in range(4 * n, 4 * n + 4):
                ps_t = pp.tile([P, P], BF16, tag="A", bufs=3, name="ps_t")
                nc.tensor.transpose(ps_t, vT_sb[:, sc * P : (sc + 1) * P], identb)
                for h in range(HPC):
                    nc.vector.tensor_copy(
                        out=v_sb[h][:, sc, 0:HS], in_=ps_t[:, h * HS : (h + 1) * HS]
                    )

        emit_p1(0)

        def emit_norm(qc):
            # 1/s rows via exp(-ln(s)), broadcast via K=1 matmuls, one
            # normalize multiply per head
            rb_ps = psA("ps_rb")
            for h in range(HPC):
                lg = work.tile([1, 512], F32, tag=f"lg{h}", bufs=2, name="lg")
                nc.scalar.activation(
                    out=lg,
                    in_=den_sb[(qc, h)],
                    func=mybir.ActivationFunctionType.Ln,
                )
                rrow = work.tile([1, 512], F32R, tag=f"rr{h}", bufs=2, name="rrow")
                nc.scalar.activation(
                    out=rrow,
                    in_=lg,
                    func=mybir.ActivationFunctionType.Exp,
                    scale=-1.0,
                )
                nc.tensor.matmul(
                    rb_ps[0:HS, h, :],
                    lhsT=onesr_sb[:, 0:HS],
                    rhs=rrow,
                    start=True,
                    stop=True,
                )
            for h in range(HPC):
                nc.vector.tensor_mul(
                    out=u2n_sb[qc][h * HS : (h + 1) * HS, :],
                    in0=u2_sb[qc][h * HS : (h + 1) * HS, :],
                    in1=rb_ps[0:HS, h, :],
                )

        # ---- phase 2: causal attention, software-pipelined ------------------
        for qc in range(NQC):
            ps_o = [
                pp.tile([P, 512], F32, tag="O", bufs=2, name=f"ps_o{h}")
                for h in range(HPC)
            ]
            nkb = 4 * (qc + 1)  # 128-wide key blocks in the causal span
            ngrp = nkb // 2

            def emit_pv(pend, nkb=nkb, ps_o=ps_o):
                pes, kbs, f0 = pend
                for h in range(HPC):
                    for j, kb in enumerate(kbs):
                        nc.tensor.matmul(
                            ps_o[h][0 : HS + 1, f0:512],
                            lhsT=v_sb[h][:, kb, :],
                            rhs=pes[h][:, j, f0:512],
                            start=(kb == 0),
                            stop=(kb == nkb - 1),
                        )

            pending = None  # exp'd logits awaiting their PV matmuls
            for g in range(ngrp):
                kbs = [2 * g, 2 * g + 1]
                # last group covers only the causal upper half of the q range
                f0 = 256 if g == ngrp - 1 else 0
                # logits for both heads, adjacent for row-group packing
                ps_att = [psA(f"ps_att{h}") for h in range(HPC)]
                for j, kb in enumerate(kbs):
                    for h in range(HPC):
                        nc.tensor.matmul(
                            ps_att[h][:, j, f0:512],
                            lhsT=qkT_sb[h * HS : (h + 1) * HS, 1, kb * P : (kb + 1) * P],
                            rhs=qkT_sb[h * HS : (h + 1) * HS, 0, qc * 512 + f0 : (qc + 1) * 512],
                            start=True,
                            stop=True,
                        )
                if pending is not None:
                    emit_pv(pending)
                pes = []
                for h in range(HPC):
                    p_exp = work.tile(
                        [P, 2, 512], BF16, tag=f"pe{h}", bufs=4, name="p_exp"
                    )
                    nc.scalar.activation(
                        out=p_exp[:, :, f0:512],
                        in_=ps_att[h][:, :, f0:512],
                        func=mybir.ActivationFunctionType.Exp,
                        scale=SCALE,
                    )
                    for j, kb in enumerate(kbs):
                        jj = kb - 4 * qc
                        if jj >= 0:  # diagonal block: causal 0/1 mask
                            off = 384 - 128 * jj
                            nc.vector.tensor_mul(
                                out=p_exp[:, j, f0:512],
                                in0=p_exp[:, j, f0:512],
                                in1=msk_sb[:, off + f0 : off + 512],
                            )
                    pes.append(p_exp)
                pending = (pes, kbs, f0)
            emit_pv(pending)

            # stash unnormalized head outputs + denominator rows; frees PSUM
            for h in range(HPC):
                nc.vector.tensor_copy(
                    out=u2_sb[qc][h * HS : (h + 1) * HS, :], in_=ps_o[h][0:HS, :]
                )
                nc.vector.tensor_copy(
                    out=den_sb[(qc, h)], in_=ps_o[h][HS : HS + 1, :]
                )
            if qc + 1 < NQC:
                emit_p1(qc + 1)
            if qc >= 1:
                emit_norm(qc - 1)

        # ---- denominator reciprocals as rows: 1/s = exp(-ln(s)) on ScalarE,
        # broadcast to all partitions with K=1 matmuls, then one multiply
        # pre-normalizes each 512-wide chunk of the attention output
        # ---- phase 3: projection over both heads (K=128), pipelined ---------
        def emit_p3_mm(sc):
            qc = sc // 4
            f = sc % 4
            slot = psA("ps_p3")
            for dc in range(2):
                nc.tensor.matmul(
                    slot[:, dc, :],
                    lhsT=u2n_sb[qc][:, f * P : (f + 1) * P],
                    rhs=wp_sb[:, dc * 512 : (dc + 1) * 512],
                    start=True,
                    stop=True,
                )
            return slot

        def emit_p3_epi(sc, slot):
            out_t = work.tile([P, 2, 512], F32, tag="out", bufs=3, name="out_t")
            nc.vector.tensor_copy(out=out_t, in_=slot)
            eng = nc.sync if sc % 2 == 0 else nc.gpsimd
            eng.dma_start(
                out=out.ap()[sc * P : (sc + 1) * P, :],
                in_=out_t.rearrange("p a b -> p (a b)"),
            )

        p3q = []
        for sc in range(NSC):
            p3q.append((sc, emit_p3_mm(sc)))
            if sc == 1:
                emit_norm(3)
            if len(p3q) > 1:
                emit_p3_epi(*p3q.pop(0))
        for item in p3q:
            emit_p3_epi(*item)

    nc.compile()
    return nc


_NC = None


def _get_nc():
    global _NC
    if _NC is None:
        _NC = _build()
    return _NC


def prepare_inputs(hidden_states, W_attn, b_attn, W_proj, b_proj):
    hs = np.asarray(hidden_states, dtype=np.float32)
    Wa = np.asarray(W_attn, dtype=np.float32)
    ba = np.asarray(b_attn, dtype=np.float32)
    Wp = np.asarray(W_proj, dtype=np.float32)

    hsT = np.ascontiguousarray(hs.T).astype(NP_BF16)
    pcol = np.arange(P)[:, None]
    ccol = np.arange(896)[None, :]
    msk = (pcol <= ccol - 384).astype(NP_BF16)

    in_maps = []
    for c in range(NCORES):
        q0 = c * CD
        wq = Wa[:, q0 : q0 + CD]
        wk = Wa[:, D + q0 : D + q0 + CD]
        wv = Wa[:, 2 * D + q0 : 2 * D + q0 + CD]
        bq = ba[q0 : q0 + CD]
        bk = ba[D + q0 : D + q0 + CD]
        bv = ba[2 * D + q0 : 2 * D + q0 + CD]
        in_maps.append(
            {
                "hsT": hsT,
                "w_qkv": np.ascontiguousarray(
                    np.concatenate([wq, wk, wv], axis=1)
                ).astype(NP_BF16),
                "b_qkv": np.ascontiguousarray(np.stack([bq, bk, bv], axis=1)).astype(
                    np.float32
                ),
                "w_p": np.ascontiguousarray(Wp[q0 : q0 + CD, :], dtype=np.float32),
                "msk": msk,
                "iden_b": np.eye(P).astype(NP_BF16),
                "ones_r": np.ones((1, P), dtype=np.float32),
                "vones": np.ones((P, NSC)).astype(NP_BF16),
            }
        )
    return in_maps


def run(inputs, trace=False):
    """Build+run the sharded kernel. Returns (full_output, BassKernelResults)."""
    in_maps = prepare_inputs(**inputs)
    nc = _get_nc()
    res = run_bass_kernel_spmd(
        nc, in_maps, core_ids=list(range(NCORES)), trace=trace
    )
    acc = np.zeros((S, D), dtype=np.float32)
    for c in range(NCORES):
        acc += res.results[c]["out"]
    acc += np.asarray(inputs["b_proj"], dtype=np.float32)
    return acc, res


def kernel(**inputs):
    out, _ = run(inputs, trace=False)
    return out


# revision 35
# speedup vs baseline: 1.1306x; 1.0016x over previous
"""Causal self-attention (S=2048, D=1024, H=16) on 8 Trainium2 NeuronCores.

Sharding: tensor-parallel over heads. Core c owns heads 2c, 2c+1:
  - computes qT/kT/vT for its 128 qkv-columns from the full hidden_states
    (contraction layouts; vT is PE-transposed back to natural [s, j]),
  - runs causal attention for its 2 heads (attT = K.Q^T blocks, exp via
    ScalarE, denominators via a ones-column in the PV matmul),
  - projects each head against its W_proj row-slice and fuses the softmax
    normalization into the projection epilogue (per-partition 1/den scales,
    denominators PE-transposed so one wide reciprocal covers all of them),
  - outputs a partial [S, D] product; the host sums the 8 partials and
    adds b_proj.

The bulk matmuls run in bf16 (1 cycle/row); the unnormalized attention
outputs, denominators and the final projection stay in float32r (fp32
storage, single-pass PE). Phase 2 is software-pipelined (logits of group g
overlap the PV matmuls of group g-1) to keep the PE dense and the HAM
clock warm; dummy matmuls paced by the input DMA keep the clock warm
during the initial load.
"""

import math
from contextlib import ExitStack

import numpy as np

import concourse.bacc as bacc
import concourse.mybir as mybir
import concourse.tile as tile
from concourse.bass_utils import run_bass_kernel_spmd

S, D, H = 2048, 1024, 16
HS = D // H  # 64 head size
P = 128
NCORES = 8
HPC = H // NCORES  # 2 heads per core
CD = HPC * HS  # 128 per-core head dims
KO = D // P  # 8 contraction tiles for the projections
NQC = S // 512  # 4 query chunks
NSC = S // P  # 16 sequence chunks of 128
SCALE = 1.0 / math.sqrt(S)

F32 = mybir.dt.float32
F32R = mybir.dt.float32r
BF16 = mybir.dt.bfloat16

try:
    import ml_dtypes

    NP_BF16 = ml_dtypes.bfloat16
except ImportError:  # pragma: no cover
    NP_BF16 = None


def _build():
    nc = bacc.Bacc(
        "TRN2", target_bir_lowering=False, debug=False, num_devices=NCORES
    )

    hsT = nc.dram_tensor("hsT", [D, S], BF16, kind="ExternalInput")
    w_qkv = nc.dram_tensor("w_qkv", [D, 3 * P], BF16, kind="ExternalInput")
    b_qkv = nc.dram_tensor("b_qkv", [P, 3], F32, kind="ExternalInput")
    w_p = nc.dram_tensor("w_p", [CD, D], F32R, kind="ExternalInput")
    msk = nc.dram_tensor("msk", [P, 896], BF16, kind="ExternalInput")
    iden_b = nc.dram_tensor("iden_b", [P, P], BF16, kind="ExternalInput")
    vones = nc.dram_tensor("vones", [P, NSC], BF16, kind="ExternalInput")
    ones_r = nc.dram_tensor("ones_r", [1, P], F32R, kind="ExternalInput")
    out = nc.dram_tensor("out", [S, D], F32, kind="ExternalOutput")

    with (
        tile.TileContext(nc) as tc,
        ExitStack() as ctx,
        nc.allow_low_precision(reason="bf16/float32r matmul pipeline"),
    ):
        const = ctx.enter_context(tc.tile_pool(name="const", bufs=1))
        work = ctx.enter_context(tc.tile_pool(name="work", bufs=2))
        pp = ctx.enter_context(tc.tile_pool(name="pp", bufs=1, space="PSUM"))

        def psA(name):  # generic 2-bank matmul target, 3 slots
            return pp.tile([P, 2, 512], F32, tag="A", bufs=3, name=name)

        # ---- loads: identity first (pre-warm), then per-o weight+hsT chunks
        identb = const.tile([P, P], BF16, tag="identb", name="identb")
        nc.sync.dma_start(out=identb, in_=iden_b.ap())
        onesr_sb = const.tile([1, P], F32R, tag="onesr", name="onesr_sb")
        nc.sync.dma_start(out=onesr_sb, in_=ones_r.ap())
        bqkv_sb = const.tile([P, 3], F32, tag="bqkv", name="bqkv_sb")
        nc.sync.dma_start(out=bqkv_sb, in_=b_qkv.ap())

        hsT_sb = const.tile([P, KO, S], BF16, tag="hsT", name="hsT_sb")
        wqkv_sb = const.tile([P, KO, 3 * P], BF16, tag="wqkv", name="wqkv_sb")
        for o in range(KO):
            nc.gpsimd.dma_start(
                out=wqkv_sb[:, o, :], in_=w_qkv.ap()[o * P : (o + 1) * P, :]
            )
            nc.sync.dma_start(
                out=hsT_sb[:, o, 0:512], in_=hsT.ap()[o * P : (o + 1) * P, 0:512]
            )
        wp_sb = const.tile([P, D], F32R, tag="wp", name="wp_sb")
        nc.sync.dma_start(out=wp_sb, in_=w_p.ap())
        msk_sb = const.tile([P, 896], BF16, tag="msk", name="msk_sb")
        nc.sync.dma_start(out=msk_sb, in_=msk.ap())
        v_sb = []
        for h in range(HPC):
            vt = const.tile([P, NSC, HS + 1], BF16, tag=f"v{h}", name=f"v{h}_sb")
            nc.sync.dma_start(out=vt[:, :, HS], in_=vones.ap())
            v_sb.append(vt)
        for n in range(1, NQC):
            for o in range(KO):
                eng = nc.sync if (n * KO + o) % 2 == 0 else nc.gpsimd
                eng.dma_start(
                    out=hsT_sb[:, o, n * 512 : (n + 1) * 512],
                    in_=hsT.ap()[o * P : (o + 1) * P, n * 512 : (n + 1) * 512],
                )

        qkT_sb = const.tile([P, 2, S], BF16, tag="qkT", name="qkT_sb")
        vT_sb = const.tile([P, S], BF16, tag="vT", name="vT_sb")
        u2_sb = [
            const.tile([P, 512], F32R, tag=f"u2_{qc}", name=f"u2_{qc}")
            for qc in range(NQC)
        ]
        u2n_sb = [
            const.tile([P, 512], F32R, tag=f"u2n_{qc}", name=f"u2n_{qc}")
            for qc in range(NQC)
        ]
        den_sb = {
            (qc, h): const.tile([1, 512], F32, tag=f"den_{qc}_{h}", name=f"den_{qc}_{h}")
            for qc in range(NQC)
            for h in range(HPC)
        }

        # ---- pre-warm the PE clock while the DMAs stream ---------------------
        # each burst consumes a freshly-arrived hsT chunk so the bursts are
        # spread across the load instead of back-to-back at t=0
        ps_w = psA("ps_w")
        for o in range(KO):
            for rep in range(6):
                nc.tensor.matmul(
                    ps_w[:, 0, :],
                    lhsT=identb,
                    rhs=hsT_sb[:, o, 0:512],
                    start=True,
                    stop=True,
                )

        # ---- phase 1: qT, kT, vT ([j, s] layout) + v transposes -------------
        # emitted per 512-chunk, interleaved with the attention chunks that
        # consume them (phase-1 is DMA-paced, phase-2 is ScalarE-bound; the
        # PE fills both phases' gaps)
        def emit_p1(n):
            for m in range(3):
                ps_qkv = psA("ps_qkv")[:, 0, :]
                for o in range(KO):
                    nc.tensor.matmul(
                        ps_qkv,
                        lhsT=wqkv_sb[:, o, m * P : (m + 1) * P],
                        rhs=hsT_sb[:, o, n * 512 : (n + 1) * 512],
                        start=(o == 0),
                        stop=(o == KO - 1),
                    )
                dst = (
                    qkT_sb[:, m, n * 512 : (n + 1) * 512]
                    if m < 2
                    else vT_sb[:, n * 512 : (n + 1) * 512]
                )
                nc.vector.tensor_scalar_add(
                    out=dst, in0=ps_qkv, scalar1=bqkv_sb[:, m : m + 1]
                )
            # transpose this n-chunk of vT into natural v layout
            for sc in range(4 * n, 4 * n + 4):
                ps_t = pp.tile([P, P], BF16, tag="A", bufs=3, name="ps_t")
                nc.tensor.transpose(ps_t, vT_sb[:, sc * P : (sc + 1) * P], identb)
                for h in range(HPC):
                    nc.vector.tensor_copy(
                        out=v_sb[h][:, sc, 0:HS], in_=ps_t[:, h * HS : (h + 1) * HS]
                    )

        emit_p1(0)

        def emit_norm(qc):
            # 1/s rows via exp(-ln(s)), broadcast via K=1 matmuls, one
            # normalize multiply per head
            rb_ps = psA("ps_rb")
            for h in range(HPC):
                lg = work.tile([1, 512], F32, tag=f"lg{h}", bufs=2, name="lg")
                nc.scalar.activation(
                    out=lg,
                    in_=den_sb[(qc, h)],
                    func=mybir.ActivationFunctionType.Ln,
                )
                rrow = work.tile([1, 512], F32R, tag=f"rr{h}", bufs=2, name="rrow")
                nc.scalar.activation(
                    out=rrow,
                    in_=lg,
                    func=mybir.ActivationFunctionType.Exp,
                    scale=-1.0,
                )
                nc.tensor.matmul(
                    rb_ps[0:HS, h, :],
                    lhsT=onesr_sb[:, 0:HS],
                    rhs=rrow,
                    start=True,
                    stop=True,
                )
            for h in range(HPC):
                nc.vector.tensor_mul(
                    out=u2n_sb[qc][h * HS : (h + 1) * HS, :],
                    in0=u2_sb[qc][h * HS : (h + 1) * HS, :],
                    in1=rb_ps[0:HS, h, :],
                )

        # ---- phase 3: projection over both heads (K=128), pipelined ---------
        def emit_p3_mm(sc):
            qc = sc // 4
            f = sc % 4
            slot = psA("ps_p3")
            for dc in range(2):
                nc.tensor.matmul(
                    slot[:, dc, :],
                    lhsT=u2n_sb[qc][:, f * P : (f + 1) * P],
                    rhs=wp_sb[:, dc * 512 : (dc + 1) * 512],
                    start=True,
                    stop=True,
                )
            return slot

        def emit_p3_epi(sc, slot):
            out_t = work.tile([P, 2, 512], F32, tag="out", bufs=3, name="out_t")
            nc.vector.tensor_copy(out=out_t, in_=slot)
            eng = nc.sync if sc % 2 == 0 else nc.gpsimd
            eng.dma_start(
                out=out.ap()[sc * P : (sc + 1) * P, :],
                in_=out_t.rearrange("p a b -> p (a b)"),
            )


        # ---- phase 2: causal attention, software-pipelined ------------------
        for qc in range(NQC):
            ps_o = [
                pp.tile([P, 512], F32, tag="O", bufs=2, name=f"ps_o{h}")
                for h in range(HPC)
            ]
            nkb = 4 * (qc + 1)  # 128-wide key blocks in the causal span
            ngrp = nkb // 2

            def emit_pv(pend, nkb=nkb, ps_o=ps_o):
                pes, kbs, f0 = pend
                for h in range(HPC):
                    for j, kb in enumerate(kbs):
                        nc.tensor.matmul(
                            ps_o[h][0 : HS + 1, f0:512],
                            lhsT=v_sb[h][:, kb, :],
                            rhs=pes[h][:, j, f0:512],
                            start=(kb == 0),
                            stop=(kb == nkb - 1),
                        )

            pending = None  # exp'd logits awaiting their PV matmuls
            for g in range(ngrp):
                kbs = [2 * g, 2 * g + 1]
                # last group covers only the causal upper half of the q range
                f0 = 256 if g == ngrp - 1 else 0
                # logits for both heads, adjacent for row-group packing
                ps_att = [psA(f"ps_att{h}") for h in range(HPC)]
                for j, kb in enumerate(kbs):
                    for h in range(HPC):
                        nc.tensor.matmul(
                            ps_att[h][:, j, f0:512],
                            lhsT=qkT_sb[h * HS : (h + 1) * HS, 1, kb * P : (kb + 1) * P],
                            rhs=qkT_sb[h * HS : (h + 1) * HS, 0, qc * 512 + f0 : (qc + 1) * 512],
                            start=True,
                            stop=True,
                        )
                if pending is not None:
                    emit_pv(pending)
                pes = []
                for h in range(HPC):
                    p_exp = work.tile(
                        [P, 2, 512], BF16, tag=f"pe{h}", bufs=4, name="p_exp"
                    )
                    nc.scalar.activation(
                        out=p_exp[:, :, f0:512],
                        in_=ps_att[h][:, :, f0:512],
                        func=mybir.ActivationFunctionType.Exp,
                        scale=SCALE,
                    )
                    for j, kb in enumerate(kbs):
                        jj = kb - 4 * qc
                        if jj >= 0:  # diagonal block: causal 0/1 mask
                            off = 384 - 128 * jj
                            nc.vector.tensor_mul(
                                out=p_exp[:, j, f0:512],
                                in0=p_exp[:, j, f0:512],
                                in1=msk_sb[:, off + f0 : off + 512],
                            )
                    pes.append(p_exp)
                pending = (pes, kbs, f0)
            emit_pv(pending)

            # stash unnormalized head outputs + denominator rows; frees PSUM
            for h in range(HPC):
                nc.vector.tensor_copy(
                    out=u2_sb[qc][h * HS : (h + 1) * HS, :], in_=ps_o[h][0:HS, :]
                )
                nc.vector.tensor_copy(
                    out=den_sb[(qc, h)], in_=ps_o[h][HS : HS + 1, :]
                )
            if qc + 1 < NQC:
                emit_p1(qc + 1)
            if qc >= 1:
                emit_norm(qc - 1)
            if qc == 2:
                for sc_e in (0, 1):
                    emit_p3_epi(sc_e, emit_p3_mm(sc_e))
            if qc == 3:
                for sc_e in (2, 3, 4, 5):
                    emit_p3_epi(sc_e, emit_p3_mm(sc_e))

        # ---- denominator reciprocals as rows: 1/s = exp(-ln(s)) on ScalarE,
        # broadcast to all partitions with K=1 matmuls, then one multiply
        # pre-normalizes each 512-wide chunk of the attention output
        p3q = []
        for sc in range(6, NSC):
            p3q.append((sc, emit_p3_mm(sc)))
            if sc == 6:
                emit_norm(3)
            if len(p3q) > 1:
                emit_p3_epi(*p3q.pop(0))
        for item in p3q:
            emit_p3_epi(*item)

    nc.compile()
    return nc


_NC = None


def _get_nc():
    global _NC
    if _NC is None:
        _NC = _build()
    return _NC


def prepare_inputs(hidden_states, W_attn, b_attn, W_proj, b_proj):
    hs = np.asarray(hidden_states, dtype=np.float32)
    Wa = np.asarray(W_attn, dtype=np.float32)
    ba = np.asarray(b_attn, dtype=np.float32)
    Wp = np.asarray(W_proj, dtype=np.float32)

    hsT = np.ascontiguousarray(hs.T).astype(NP_BF16)
    pcol = np.arange(P)[:, None]
    ccol = np.arange(896)[None, :]
    msk = (pcol <= ccol - 384).astype(NP_BF16)

    in_maps = []
    for c in range(NCORES):
        q0 = c * CD
        wq = Wa[:, q0 : q0 + CD]
        wk = Wa[:, D + q0 : D + q0 + CD]
        wv = Wa[:, 2 * D + q0 : 2 * D + q0 + CD]
        bq = ba[q0 : q0 + CD]
        bk = ba[D + q0 : D + q0 + CD]
        bv = ba[2 * D + q0 : 2 * D + q0 + CD]
        in_maps.append(
            {
                "hsT": hsT,
                "w_qkv": np.ascontiguousarray(
                    np.concatenate([wq, wk, wv], axis=1)
                ).astype(NP_BF16),
                "b_qkv": np.ascontiguousarray(np.stack([bq, bk, bv], axis=1)).astype(
                    np.float32
                ),
                "w_p": np.ascontiguousarray(Wp[q0 : q0 + CD, :], dtype=np.float32),
                "msk": msk,
                "iden_b": np.eye(P).astype(NP_BF16),
                "ones_r": np.ones((1, P), dtype=np.float32),
                "vones": np.ones((P, NSC)).astype(NP_BF16),
            }
        )
    return in_maps


def run(inputs, trace=False):
    """Build+run the sharded kernel. Returns (full_output, BassKernelResults)."""
    in_maps = prepare_inputs(**inputs)
    nc = _get_nc()
    res = run_bass_kernel_spmd(
        nc, in_maps, core_ids=list(range(NCORES)), trace=trace
    )
    acc = np.zeros((S, D), dtype=np.float32)
    for c in range(NCORES):
        acc += res.results[c]["out"]
    acc += np.asarray(inputs["b_proj"], dtype=np.float32)
    return acc, res


def kernel(**inputs):
    out, _ = run(inputs, trace=False)
    return out
